# revision 1
# baseline (speedup 1.0000x reference)
"""Causal self-attention (B=4, T=2048, C=1024, H=16) on 8 TRN2 NeuronCores.

Sharding:
  - QKV + attention: tensor-parallel over heads (2 heads/core, all batches).
  - Output projection: data-parallel over tokens (1024 tokens/core),
    connected by one AllToAll per half-batch (512 KB/core each).

Layouts (everything feeds the PE in natural form, host pre-transposes):
  - host passes xT = x^T [C, B*T]; per-core W_attn q/k/v slices transposed
    [C, 128]; W_proj^T [C, C] replicated.
  - QKV computes qkvT [qkv_dim, tokens] (tokens moving, N=512, fp32r).
  - attention in S^T layout: S^T[s, t] = K @ Q^T per 128-key tile, both
    heads side by side in one [128, 1024] PSUM tile; one exp per key-tile
    on ScalarE with 1/sqrt(D) folded into the activation scale; causal
    handled by trimming block ranges + a [128,128] additive tri-mask on
    diagonal blocks.
  - softmax sums via a ones-column appended to PE-transposed V (bf16);
    P@V in bf16; division by the sum via DVE reciprocal + gpsimd
    partition_broadcast + one DVE multiply.
  - projection: lhsT = y^T tiles from the AllToAll, rhs = W_proj^T ->
    output lands token-major, DMA'd straight out.
"""

import sys

sys.path.insert(0, "/opt/trn_rl_repo")

import numpy as np

import concourse.bass as bass
import concourse.bacc as bacc
import concourse.mybir as mybir
import concourse.tile as tile
from concourse.bass_utils import run_bass_kernel_spmd

N_CORES = 8
B, T, C = 4, 2048, 1024
H, D = 16, 64
HPC = H // N_CORES          # heads per core = 2
BT = B * T                  # 8192 flattened tokens
QB = 512                    # query block
SB = 128                    # key tile
NQB = T // QB               # 4 query blocks per batch
NSB = T // SB               # 16 key tiles per batch
TOKS = BT // N_CORES        # 1024 output tokens per core
TPB = 256                   # tokens per (core, batch)

F32 = mybir.dt.float32
F32R = mybir.dt.float32r
BF16 = mybir.dt.bfloat16
EXP = mybir.ActivationFunctionType.Exp

RUN_KWARGS: dict = {}
LAST_RESULTS = None

_PROGRAM = None


def _build_program():
    nc = bacc.Bacc(num_devices=N_CORES)

    xT = nc.declare_dram_parameter("xT", [C, BT], F32R, isOutput=False)
    wq = nc.declare_dram_parameter("wq", [C, 128], F32R, isOutput=False)
    wk = nc.declare_dram_parameter("wk", [C, 128], F32R, isOutput=False)
    wv = nc.declare_dram_parameter("wv", [C, 128], F32R, isOutput=False)
    wp = nc.declare_dram_parameter("wp", [C, C], F32R, isOutput=False)
    ntri = nc.declare_dram_parameter("ntri", [128, 128], F32, isOutput=False)
    ident = nc.declare_dram_parameter("ident", [128, 128], F32, isOutput=False)
    ones = nc.declare_dram_parameter("ones", [128, 1], F32, isOutput=False)
    one64 = nc.declare_dram_parameter("one64", [1, 64], F32R, isOutput=False)
    out_ext = nc.declare_dram_parameter("out", [TOKS, C], F32, isOutput=True)

    # A2A bounce buffers, one pair per batch (payload stays f32r bits)
    sends = [nc.dram_tensor(f"send{b}", [N_CORES * 128, TPB], F32R) for b in range(B)]
    recvs = [nc.dram_tensor(f"recv{b}", [N_CORES * 128, TPB], F32R) for b in range(B)]

    with tile.TileContext(nc) as tc:
        with (
            tc.tile_pool(name="const", bufs=1) as constp,
            tc.tile_pool(name="wgt", bufs=1) as wgtp,
            tc.tile_pool(name="qk", bufs=1) as qkp,
            tc.tile_pool(name="vt", bufs=4) as vtp,
            tc.tile_pool(name="vaug", bufs=4) as vaugp,
            tc.tile_pool(name="xt", bufs=9) as xtp,
            tc.tile_pool(name="pp", bufs=6) as ppool,
            tc.tile_pool(name="ysb", bufs=2) as ysbp,
            tc.tile_pool(name="osb", bufs=2) as osbp,
            tc.tile_pool(name="rv", bufs=10) as rvp,
            tc.tile_pool(name="sc", bufs=3) as scp,
            tc.tile_pool(name="ps", bufs=2, space="PSUM") as psp,       # qkv/transpose/proj
            tc.tile_pool(name="sps", bufs=2, space="PSUM") as sps,      # S^T [128,1024]
            tc.tile_pool(name="yaps", bufs=2, space="PSUM") as yaps,    # y_aug [65,512]
        ):
            # ---------------- constants + attention weights ----------------
            ntri_s = constp.tile([128, 128], F32, tag="ntri")
            nc.sync.dma_start(out=ntri_s[:], in_=ntri[:])
            ident_s = constp.tile([128, 128], BF16, tag="ident")
            nc.gpsimd.dma_start(out=ident_s[:], in_=ident[:])
            ones_s = constp.tile([128, 1], F32, tag="ones")
            nc.sync.dma_start(out=ones_s[:], in_=ones[:])
            one64_s = constp.tile([1, 64], F32R, tag="one64")
            nc.sync.dma_start(out=one64_s[:], in_=one64[:])

            wq_s = wgtp.tile([128, 8 * 128], F32R, tag="wq")
            wk_s = wgtp.tile([128, 8 * 128], F32R, tag="wk")
            wv_s = wgtp.tile([128, 8 * 128], F32R, tag="wv")
            for dst, src in ((wq_s, wq), (wk_s, wk), (wv_s, wv)):
                nc.sync.dma_start(
                    out=dst[:].rearrange("p (c d) -> p c d", c=8),
                    in_=src[:].rearrange("(c p) d -> p c d", p=128),
                )
            # wp_s is loaded lazily (emitted after phase1(0)) so it doesn't
            # delay the first xt loads on the SWDGE queue.
            wp_s = wgtp.tile([128, 8 * 1024], F32R, tag="wp")

            def load_wp():
                nc.sync.dma_start(
                    out=wp_s[:].rearrange("p (c d) -> p c d", c=8),
                    in_=wp[:].rearrange("(c p) d -> p c d", p=128),
                )

            qT = qkp.tile([128, BT], F32R, tag="qT")
            kT = qkp.tile([128, BT], F32R, tag="kT")

            vaug = {}   # (b, h) -> [128, NSB*65] bf16
            vt_hb = {}  # b -> [vt tiles per head]

            def qkv_gen(b, tb):
                """QKV for one 512-token block (generator: yields between
                small PE chunks so attention can interleave)."""
                if tb == 0:
                    vt_hb[b] = []
                    for h in range(HPC):
                        vth = vtp.tile([65, T], BF16, tag="vt", name=f"vt{b}_{h}")
                        nc.gpsimd.memset(vth[64:65, :], 1.0)
                        vt_hb[b].append(vth)
                base = b * T + tb * QB
                xts = []
                for c in range(8):
                    xt_t = xtp.tile([128, QB], F32R, tag="xt")
                    nc.sync.dma_start(
                        out=xt_t[:], in_=xT[c * 128 : (c + 1) * 128, base : base + QB]
                    )
                    xts.append(xt_t)
                yield
                pq = psp.tile([128, QB], F32, tag="ps")
                pk = psp.tile([128, QB], F32, tag="ps")
                for c in range(8):
                    st = dict(start=(c == 0), stop=(c == 7))
                    nc.tensor.matmul(pq[:], wq_s[:, c * 128 : (c + 1) * 128], xts[c][:], **st)
                    nc.tensor.matmul(pk[:], wk_s[:, c * 128 : (c + 1) * 128], xts[c][:], **st)
                    yield
                nc.vector.tensor_copy(qT[:, base : base + QB], pq[:])
                nc.vector.tensor_copy(kT[:, base : base + QB], pk[:])
                yield
                pv = psp.tile([128, QB], F32, tag="ps")
                for c in range(8):
                    nc.tensor.matmul(
                        pv[:], wv_s[:, c * 128 : (c + 1) * 128], xts[c][:],
                        start=(c == 0), stop=(c == 7),
                    )
                    if c % 2 == 1:
                        yield
                tloc = tb * QB
                for h in range(HPC):
                    nc.vector.tensor_copy(
                        vt_hb[b][h][0:64, tloc : tloc + QB], pv[h * 64 : (h + 1) * 64, :]
                    )
                yield

            def vaug_gen(b, h):
                """PE-transpose v^T into V_aug (+ ones col) for one head."""
                va = vaugp.tile([128, NSB * 65], BF16, tag="vaug", name=f"va{b}_{h}")
                vaug[(b, h)] = va
                vth = vt_hb[b][h]
                for j in range(NSB):
                    tr = psp.tile([128, 65], BF16, tag="ps", name="tr")
                    nc.tensor.transpose(tr[:], vth[:, j * SB : (j + 1) * SB], ident_s[0:65, 0:65])
                    nc.vector.tensor_copy(va[:, j * 65 : (j + 1) * 65], tr[:])
                    if j % 2 == 1:
                        yield

            def attn_unit(b, i, feed):
                """One query block (both heads together in S^T / exp)."""
                ya = [yaps.tile([65, QB], F32, tag="yaug", name=f"ya{h}") for h in range(HPC)]
                jmax = 4 * (i + 1)
                pend = []

                def do_pv(last):
                    pp3, pr, pj = pend.pop(0)
                    for h in range(HPC):
                        nc.tensor.matmul(
                            ya[h][:, pr:QB],
                            vaug[(b, h)][:, pj * 65 : pj * 65 + 65],
                            pp3[:, h, pr:QB],
                            start=(pj == 0), stop=last,
                        )

                for j in range(jmax):
                    diag = j >= 4 * i
                    r = SB * j - QB * i if diag else 0
                    sp = sps.tile([128, 2 * QB], F32, tag="sp")
                    sp3 = sp[:].rearrange("p (h t) -> p h t", h=2)
                    for h in range(HPC):
                        nc.tensor.matmul(
                            sp3[:, h, r:QB],
                            kT[h * 64 : (h + 1) * 64, b * T + j * SB : b * T + (j + 1) * SB],
                            qT[h * 64 : (h + 1) * 64, b * T + i * QB + r : b * T + (i + 1) * QB],
                            start=True, stop=True,
                        )
                        if diag:
                            nc.vector.tensor_add(
                                sp3[:, h, r : r + 128], sp3[:, h, r : r + 128], ntri_s[:]
                            )
                    pt = ppool.tile([128, 2 * QB], BF16, tag="P")
                    pt3 = pt[:].rearrange("p (h t) -> p h t", h=2)
                    nc.scalar.activation(pt3[:, :, r:QB], sp3[:, :, r:QB], EXP, scale=0.125)
                    pend.append((pt3, r, j))
                    if len(pend) > 2:
                        do_pv(False)
                    feed(1)
                while len(pend) > 1:
                    do_pv(False)
                do_pv(True)
                # softmax division + scatter into the A2A send buffer
                for h in range(HPC):
                    rc = scp.tile([1, QB], F32, tag="recip")
                    nc.vector.reciprocal(rc[:], ya[h][64:65, :])
                    bc = scp.tile([64, QB], F32, tag="bcast")
                    nc.gpsimd.partition_broadcast(bc[:], rc[:])
                    yt = ysbp.tile([64, QB], F32R, tag="ysb")
                    nc.vector.tensor_mul(yt[:], ya[h][0:64, :], bc[:])
                    for q in range(2):
                        m = 2 * i + q
                        nc.sync.dma_start(
                            out=sends[b][m * 128 + h * 64 : m * 128 + (h + 1) * 64, :],
                            in_=yt[:, q * 256 : (q + 1) * 256],
                        )

            def a2a(b):
                nc.gpsimd.collective_compute(
                    "AllToAll",
                    mybir.AluOpType.bypass,
                    replica_groups=[list(range(N_CORES))],
                    ins=[sends[b][:]],
                    outs=[recvs[b][:]],
                )

            def proj_gen(b):
                """Projection for this core's 256 tokens of batch b."""
                rvs = []
                for c in range(8):
                    rt = rvp.tile([128, TPB], F32R, tag="rv")
                    nc.sync.dma_start(out=rt[:], in_=recvs[b][c * 128 : (c + 1) * 128, :])
                    rvs.append(rt)
                yield
                for tt in range(2):
                    ob = osbp.tile([128, C], F32, tag="osb")
                    for co in range(2):
                        pj = psp.tile([128, 512], F32, tag="ps", name="pj")
                        for c in range(8):
                            nc.tensor.matmul(
                                pj[:],
                                rvs[c][:, tt * 128 : (tt + 1) * 128],
                                wp_s[:, c * 1024 + co * 512 : c * 1024 + (co + 1) * 512],
                                start=(c == 0), stop=(c == 7),
                            )
                            if c % 2 == 1:
                                yield
                        nc.vector.tensor_copy(ob[:, co * 512 : (co + 1) * 512], pj[:])
                    row = b * 256 + tt * 128
                    nc.sync.dma_start(out=out_ext[row : row + 128, :], in_=ob[:])
                yield

            # ---------------- emission schedule ----------------
            from collections import deque

            class Feeder:
                def __init__(self):
                    self.q = deque()
                    self.added = 0
                    self.finished = 0

                def add(self, *gens):
                    self.q.extend(gens)
                    self.added += len(gens)
                    return self.added

                def feed(self, n=1):
                    done = 0
                    while self.q and done < n:
                        try:
                            next(self.q[0])
                            done += 1
                        except StopIteration:
                            self.q.popleft()
                            self.finished += 1
                    return done

                def drain_to(self, mark):
                    while self.finished < mark and self.q:
                        self.feed(64)

                def drain(self):
                    while self.feed(64):
                        pass

            def phase1_gens(b):
                return [qkv_gen(b, tb) for tb in range(4)] + [
                    vaug_gen(b, h) for h in range(HPC)
                ]

            feeder = Feeder()

            def phase2(b):
                """4 attention units; fillers fed at key-tile granularity."""
                for i in range(NQB):
                    attn_unit(b, i, feeder.feed)
                a2a(b)
                feeder.add(proj_gen(b))

            m0 = feeder.add(*phase1_gens(0))
            feeder.drain_to(m0)
            load_wp()
            feeder.add(*phase1_gens(1))
            m1 = feeder.added
            phase2(0)
            feeder.drain_to(m1)
            feeder.add(*phase1_gens(2))
            m2 = feeder.added
            phase2(1)
            feeder.drain_to(m2)
            feeder.add(*phase1_gens(3))
            m3 = feeder.added
            phase2(2)
            feeder.drain_to(m3)
            phase2(3)
            feeder.drain()

    nc.finalize()
    return nc


def _f32r(a):
    """Round fp32 to the fp32r (tf32-like, 11-bit mantissa) bit pattern the
    PE expects, so plain (no-cast) DMAs can feed fp32r matmuls."""
    a = np.ascontiguousarray(a, dtype=np.float32)
    u = a.view(np.uint32)
    r = (u + 0x7FF + ((u >> 12) & 1)) & np.uint32(0xFFFFF000)
    return r.view(np.float32)


def _prep_inputs(x, W_attn, b_attn, W_proj, b_proj):
    x = np.asarray(x, dtype=np.float32)
    W_attn = np.asarray(W_attn, dtype=np.float32)
    W_proj = np.asarray(W_proj, dtype=np.float32)

    xT = _f32r(x.reshape(BT, C).T)                         # [C, BT]
    wpT = _f32r(W_proj.T)                                  # [C, C]

    s = np.arange(128)[:, None]
    t = np.arange(128)[None, :]
    ntri = np.where(t >= s, 0.0, -1e9).astype(np.float32)  # valid: key <= query
    ident = np.eye(128, dtype=np.float32)
    one64 = np.ones((1, 64), dtype=np.float32)
    ones = np.ones((128, 1), dtype=np.float32)

    in_maps = []
    for k in range(N_CORES):
        r0 = k * HPC * D                                   # 128*k
        wq_k = _f32r(W_attn[r0 : r0 + 128, :].T)
        wk_k = _f32r(W_attn[C + r0 : C + r0 + 128, :].T)
        wv_k = _f32r(W_attn[2 * C + r0 : 2 * C + r0 + 128, :].T)
        in_maps.append(
            {
                "xT": xT,
                "wq": wq_k,
                "wk": wk_k,
                "wv": wv_k,
                "wp": wpT,
                "ntri": ntri,
                "ident": ident,
                "ones": ones,
                "one64": one64,
            }
        )
    return in_maps


def kernel(x, W_attn, b_attn, W_proj, b_proj):
    global _PROGRAM, LAST_RESULTS
    if _PROGRAM is None:
        _PROGRAM = _build_program()
    nc = _PROGRAM

    in_maps = _prep_inputs(x, W_attn, b_attn, W_proj, b_proj)
    res = run_bass_kernel_spmd(nc, in_maps, list(range(N_CORES)), **RUN_KWARGS)
    LAST_RESULTS = res

    out = np.empty((B, T, C), dtype=np.float32)
    for k in range(N_CORES):
        ok = res.results[k]["out"]                         # [TOKS, C]
        for b in range(B):
            out[b, k * TPB : (k + 1) * TPB, :] = ok[b * TPB : (b + 1) * TPB, :]
    return out



# revision 20
# speedup vs baseline: 1.2613x; 1.2613x over previous
"""Causal self-attention (B=4, T=2048, C=1024, H=16) on 8 TRN2 NeuronCores.

Sharding: tensor-parallel over heads (2 heads/core) for QKV+attention;
data-parallel over tokens for the output projection, connected by one
AllToAll per batch (fp8 hi/lo payload).

fp8 DoubleRow strategy (cost model: DR = 0.5 cycles/moving-row, two
contraction tiles per instruction):
  - QKV q/k: 3-term DR over c-tile pairs:
      [Whi(c0),Whi(c1)]x[xhi(c0),xhi(c1)] + [Whi]x[xlo pair] + [Wlo]x[xhi pair]
    x is split hi/lo fp8 on the host; W pre-scaled by 32 (fp8 subnormal
    avoidance) and split hi/lo.  Result PSUM holds 32*q exactly-ish.
  - v: token-major (stationary x pair, moving Wv) -> vaug needs NO PE
    transposes; per c-tile: [xhi,xlo]x[Wvhi dup] + per c-pair [xhi,xhi]x[Wvlo].
  - S: bf16 (qT/kT evicted as 32q/32k bf16), full-rate 1 cycle/row.
  - causal mask: one PE matmul per diag block (-1e9*I stationary,
    strict-lower ones moving, broadcast over both heads).
  - exp: ACT, scale 2^-13 (=0.125/32^2), bias -3 (global softmax shift,
    exact), fp8 P out; queries 0:128 of the first diag tile use bias 0.
  - PV: per key-tile single DR: [vhi_j, vlo_j] x [P_j broadcast-dup]
    -> V exact, P fp8.  ya accumulates [65, 512] (65th row = 32-scaled
    ones column -> softmax sums).
  - proj: per c-tile [yhi,ylo]x[Wp_hi dup] + per c-pair [yhi,yhi]x[Wp_lo],
    y split hi/lo at eviction, A2A carries both fp8 halves.
Schedule: all projection work deferred to the end so proj(0..2) hides
A2A(3); output DMA'd straight from PSUM.
"""

import sys

sys.path.insert(0, "/opt/trn_rl_repo")

import numpy as np
import ml_dtypes

import concourse.bass as bass
import concourse.bacc as bacc
import concourse.mybir as mybir
import concourse.tile as tile
from concourse.bass_utils import run_bass_kernel_spmd

N_CORES = 8
B, T, C = 4, 2048, 1024
H, D = 16, 64
HPC = H // N_CORES          # 2 heads per core
BT = B * T                  # 8192 tokens
QB = 512                    # query block
SB = 128                    # key tile
NQB = T // QB               # 4 query blocks per batch
NSB = T // SB               # 16 key tiles per batch
TOKS = BT // N_CORES        # 1024 output tokens per core
TPB = 256                   # tokens per (core, batch)
SC = 32.0                   # weight/value scale (power of 2)
ESCALE = 0.125 / (SC * SC)  # exp scale on 32q.32k logits = 2^-13
SHIFT = 3.0                 # global softmax shift (exact for softmax)

F32 = mybir.dt.float32
BF16 = mybir.dt.bfloat16
F8 = mybir.dt.float8e4
EXP = mybir.ActivationFunctionType.Exp
DR = mybir.MatmulPerfMode.DoubleRow
E4 = ml_dtypes.float8_e4m3
BF = ml_dtypes.bfloat16

RUN_KWARGS: dict = {}
LAST_RESULTS = None
_PROGRAM = None


def _build_program():
    nc = bacc.Bacc(num_devices=N_CORES)

    xhi = nc.declare_dram_parameter("xhi", [C, BT], F8, isOutput=False)
    xlo = nc.declare_dram_parameter("xlo", [C, BT], F8, isOutput=False)
    wq_hi = nc.declare_dram_parameter("wq_hi", [128, 8 * 128], F8, isOutput=False)
    wq_lo = nc.declare_dram_parameter("wq_lo", [128, 8 * 128], F8, isOutput=False)
    wk_hi = nc.declare_dram_parameter("wk_hi", [128, 8 * 128], F8, isOutput=False)
    wk_lo = nc.declare_dram_parameter("wk_lo", [128, 8 * 128], F8, isOutput=False)
    wv_hid = nc.declare_dram_parameter("wv_hid", [128, 8 * 2 * 128], F8, isOutput=False)
    wv_lo = nc.declare_dram_parameter("wv_lo", [128, 8 * 128], F8, isOutput=False)
    wp_hid = nc.declare_dram_parameter("wp_hid", [128, 8 * 2 * 1024], F8, isOutput=False)
    wp_lo = nc.declare_dram_parameter("wp_lo", [128, 8 * 1024], F8, isOutput=False)
    mska = nc.declare_dram_parameter("mska", [128, 128], BF16, isOutput=False)  # -1e9*I
    mskb = nc.declare_dram_parameter("mskb", [128, 128], BF16, isOutput=False)  # [p,t]=1 if t<p
    out_ext = nc.declare_dram_parameter("out", [TOKS, C], F32, isOutput=True)
    import os
    DBG = bool(os.environ.get("BASS_DEBUG_DUMP"))
    if DBG:
        dbg_q = nc.declare_dram_parameter("dbg_q", [128, 1024], BF16, isOutput=True)
        dbg_k = nc.declare_dram_parameter("dbg_k", [128, 1024], BF16, isOutput=True)
        dbg_va = nc.declare_dram_parameter("dbg_va", [128, NSB * 256], F8, isOutput=True)
        dbg_p = nc.declare_dram_parameter("dbg_p", [128, 1024], F8, isOutput=True)
        dbg_yt = nc.declare_dram_parameter("dbg_yt", [64, 512], F32, isOutput=True)
        dbg_rv = nc.declare_dram_parameter("dbg_rv", [128, 4096], F8, isOutput=True)
        dbg_yh = nc.declare_dram_parameter("dbg_yh", [64, 512], F8, isOutput=True)
        dbg_yl = nc.declare_dram_parameter("dbg_yl", [64, 512], F8, isOutput=True)
        dbg_sd = nc.declare_dram_parameter("dbg_sd", [128, 4096], F8, isOutput=True)

    # A2A bounce buffers: rows 0:1024 = yhi (8 m-chunks x 128), 1024:2048 = ylo
    sends = [nc.dram_tensor(f"send{b}", [2 * N_CORES * 128, TPB], F8) for b in range(B)]
    recvs = [nc.dram_tensor(f"recv{b}", [2 * N_CORES * 128, TPB], F8) for b in range(B)]

    with tile.TileContext(nc) as tc:
        with (
            tc.tile_pool(name="const", bufs=1) as constp,
            tc.tile_pool(name="wgt", bufs=1) as wgtp,
            tc.tile_pool(name="qk", bufs=1) as qkp,
            tc.tile_pool(name="xt", bufs=3) as xtp,
            tc.tile_pool(name="vaug", bufs=4) as vaugp,
            tc.tile_pool(name="pp", bufs=6) as ppool,
            tc.tile_pool(name="sc", bufs=4) as scp,
            tc.tile_pool(name="ysb", bufs=4) as ysbp,
            tc.tile_pool(name="rv", bufs=4) as rvp,
            tc.tile_pool(name="ps", bufs=2, space="PSUM") as psp,     # qkv/v/proj (1 bank tiles)
            tc.tile_pool(name="sps", bufs=2, space="PSUM") as sps,    # S^T [128, 2*512] f32
            tc.tile_pool(name="yaps", bufs=2, space="PSUM") as yaps,  # ya [65, 512] f32
        ):
            # ---------------- constants + weights ----------------
            mska_s = constp.tile([128, 128], BF16, tag="mska")
            nc.sync.dma_start(out=mska_s[:], in_=mska[:])
            mskb_s = constp.tile([128, 128], BF16, tag="mskb")
            nc.sync.dma_start(out=mskb_s[:], in_=mskb[:])
            bias3 = constp.tile([128, 1], F32, tag="bias3")
            nc.gpsimd.memset(bias3[:], -SHIFT)

            wq_hi_s = wgtp.tile([128, 1024], F8, tag="wqh")
            wq_lo_s = wgtp.tile([128, 1024], F8, tag="wql")
            wk_hi_s = wgtp.tile([128, 1024], F8, tag="wkh")
            wk_lo_s = wgtp.tile([128, 1024], F8, tag="wkl")
            wv_hid_s = wgtp.tile([128, 2048], F8, tag="wvh")
            wv_lo_s = wgtp.tile([128, 1024], F8, tag="wvl")
            for dst, src in (
                (wq_hi_s, wq_hi), (wq_lo_s, wq_lo),
                (wk_hi_s, wk_hi), (wk_lo_s, wk_lo),
                (wv_hid_s, wv_hid), (wv_lo_s, wv_lo),
            ):
                nc.sync.dma_start(out=dst[:], in_=src[:])

            # wp loaded lazily after phase1(0) is emitted
            wp_hid_s = wgtp.tile([128, 16384], F8, tag="wph")
            wp_lo_s = wgtp.tile([128, 8192], F8, tag="wpl")

            def load_wp():
                nc.sync.dma_start(out=wp_hid_s[:], in_=wp_hid[:])
                nc.sync.dma_start(out=wp_lo_s[:], in_=wp_lo[:])

            qT = qkp.tile([128, BT], BF16, tag="qT")   # 32*q, bf16
            kT = qkp.tile([128, BT], BF16, tag="kT")

            vaug = {}  # (b, h) -> [128, NSB*130] F8 (per key tile: hi 65 | lo 65)

            def qkv_gen(b, tb):
                """QKV for one 512-token block: q,k 3-term DR feature-major;
                v token-major (no transposes). Yields between PE chunks."""
                if tb == 0:
                    for h in range(HPC):
                        va = vaugp.tile([128, NSB * 256], F8, tag="vaug",
                                        name=f"va{b}_{h}")
                        vaug[(b, h)] = va
                        # per-j slot layout (256 wide): hi slot [v 0:64 | ones@64 |
                        # zero pad 65:128], lo slot [v 128:192 | zero pad 192:256]
                        # (zero pads keep the unused ya rows 65:127 finite)
                        va3 = va[:].rearrange("p (j c) -> p j c", j=NSB)
                        nc.gpsimd.memset(va3[:, :, 64:128], 0.0)
                        nc.gpsimd.memset(va3[:, :, 192:256], 0.0)
                        nc.gpsimd.memset(va3[:, :, 64:65], SC)
                base = b * T + tb * QB
                xt = xtp.tile([128, 2 * 8 * QB], F8, tag="xt")  # hi: c*512, lo: 4096+c*512
                nc.sync.dma_start(
                    out=xt[:, 0:4096].rearrange("p (c t) -> p c t", c=8),
                    in_=xhi[:, base : base + QB].rearrange("(c p) t -> p c t", p=128),
                )
                nc.scalar.dma_start(
                    out=xt[:, 4096:8192].rearrange("p (c t) -> p c t", c=8),
                    in_=xlo[:, base : base + QB].rearrange("(c p) t -> p c t", p=128),
                )
                yield
                # ---- q, k: 3-term DR over c-pairs, halves of 256 tokens ----
                # xt4[p, l, c, t]: l = hi/lo, c = 8 chunks, t = 512 tokens
                xt4 = xt[:].rearrange("p (l c t) -> p l c t", l=2, c=8)
                pq = psp.tile([128, QB], F32, tag="ps", name="pq")
                pk = psp.tile([128, QB], F32, tag="ps", name="pk")
                for out_ps, whi, wlo in ((pq, wq_hi_s, wq_lo_s), (pk, wk_hi_s, wk_lo_s)):
                    for hf in range(2):
                        ts = hf * 256
                        for p in range(4):
                            lw_hi = whi[:, p * 256 : (p + 1) * 256].rearrange(
                                "k (two m) -> k two m", two=2)
                            lw_lo = wlo[:, p * 256 : (p + 1) * 256].rearrange(
                                "k (two m) -> k two m", two=2)
                            # x pair APs: [128, (c pair), 256]
                            xh = xt4[:, 0, 2 * p : 2 * p + 2, ts : ts + 256]
                            xl = xt4[:, 1, 2 * p : 2 * p + 2, ts : ts + 256]
                            first = p == 0 and hf == 0
                            last = p == 3
                            nc.tensor.matmul(out_ps[:, ts : ts + 256], lw_hi, xh,
                                             start=first, stop=False, perf_mode=DR)
                            nc.tensor.matmul(out_ps[:, ts : ts + 256], lw_hi, xl,
                                             start=False, stop=False, perf_mode=DR)
                            nc.tensor.matmul(out_ps[:, ts : ts + 256], lw_lo, xh,
                                             start=False, stop=last, perf_mode=DR)
                        yield
                nc.vector.tensor_copy(qT[:, base : base + QB], pq[:])
                nc.vector.tensor_copy(kT[:, base : base + QB], pk[:])
                yield
                # ---- v: token-major, 4 key tiles of 128 tokens ----
                # per c-tile: lhsT = (xhi(c), xlo(c)) pair -> exact x; rhs = Wv_hi dup
                # per c-pair: lhsT = (xhi(c0), xhi(c1)); rhs = (Wv_lo(c0), Wv_lo(c1))
                wv8 = wv_lo_s[:].rearrange("k (c m) -> k c m", c=8)
                pv = psp.tile([128, 4 * 128], F32, tag="ps", name="pv")  # (tok-tile, feat)
                for tt in range(2):  # two double-tile chunks to bound inst burst
                    for ttt in range(2):
                        t4 = tt * 2 + ttt
                        ts = t4 * 128
                        reg = pv[:, t4 * 128 : (t4 + 1) * 128]
                        for c in range(8):
                            xpair = xt4[:, :, c, ts : ts + 128]  # [128, 2(l), 128]
                            wvd = wv_hid_s[:, c * 256 : (c + 1) * 256].rearrange(
                                "k (two m) -> k two m", two=2)
                            nc.tensor.matmul(reg, xpair, wvd,
                                             start=(c == 0 and t4 == 0),
                                             stop=False, perf_mode=DR)
                        for p in range(4):
                            xhp = xt4[:, 0, 2 * p : 2 * p + 2, ts : ts + 128]
                            wvl = wv8[:, 2 * p : 2 * p + 2, :]
                            nc.tensor.matmul(reg, xhp, wvl,
                                             start=False, stop=(p == 3), perf_mode=DR)
                    yield
                # evictions: vhi (DVE copy), vlo (DVE mixed sub)
                for t4 in range(4):
                    j = tb * 4 + t4
                    reg = pv[:, t4 * 128 : (t4 + 1) * 128]
                    for h in range(HPC):
                        va = vaug[(b, h)]
                        hslice = reg[:, h * 64 : (h + 1) * 64]
                        nc.vector.tensor_copy(va[:, j * 256 : j * 256 + 64], hslice)
                        nc.vector.tensor_sub(va[:, j * 256 + 128 : j * 256 + 192],
                                             hslice, va[:, j * 256 : j * 256 + 64])
                yield

            def attn_unit(b, i, feed):
                """One query block: S (bf16) + PE mask + exp (fp8, shift) +
                PV single-tile DR + softmax normalize + A2A send staging."""
                ya = [yaps.tile([128, QB], F32, tag="yaug", name=f"ya{h}")
                      for h in range(HPC)]
                jmax = 4 * (i + 1)
                pend = []
                emitted = [[] for _ in range(HPC)]  # (j, r) per head, for flags

                def do_pv(last):
                    # PSUM start=True poisons the whole 2KB bank (pending-zero),
                    # so ONLY the very first inst per ya bank may set it; later
                    # insts overwrite-on-first-touch of still-pending bytes.
                    pt3, r, j = pend.pop(0)
                    for h in range(HPC):
                        va = vaug[(b, h)]
                        lhs = va[:, j * 256 : j * 256 + 256].rearrange(
                            "p (two m) -> p two m", two=2)
                        a = r
                        while a < QB:
                            bnd = min(a + 256, QB)
                            rhs = pt3[:, h, a:bnd].unsqueeze(1).broadcast_to(
                                [128, 2, bnd - a])
                            is_last = last and bnd == QB
                            nc.tensor.matmul(ya[h][:, a:bnd], lhs, rhs,
                                             start=(j == 0 and a == 0),
                                             stop=is_last, perf_mode=DR)
                            a = bnd

                for j in range(jmax):
                    diag = j >= 4 * i
                    r = SB * j - QB * i if diag else 0
                    sp = sps.tile([128, 2 * QB], F32, tag="sp")
                    sp3 = sp[:].rearrange("p (h t) -> p h t", h=2)
                    for h in range(HPC):
                        nc.tensor.matmul(
                            sp3[:, h, r:QB],
                            kT[h * 64 : (h + 1) * 64, b * T + j * SB : b * T + (j + 1) * SB],
                            qT[h * 64 : (h + 1) * 64, b * T + i * QB + r : b * T + (i + 1) * QB],
                            start=True, stop=True,
                        )
                    if diag:
                        rhsm = mskb_s[:].unsqueeze(1).broadcast_to([128, 2, 128])
                        outm = sp3[:, :, r : r + 128]
                        nc.tensor.matmul(outm, mska_s[:], rhsm, start=False, stop=True,
                                         skip_group_check=True)
                    pt = ppool.tile([128, 2 * QB], F8, tag="P")
                    pt3 = pt[:].rearrange("p (h t) -> p h t", h=2)
                    if i == 0 and j == 0:
                        nc.scalar.activation(pt3[:, :, 0:128], sp3[:, :, 0:128],
                                             EXP, scale=ESCALE, bias=0.0)
                        nc.scalar.activation(pt3[:, :, 128:QB], sp3[:, :, 128:QB],
                                             EXP, scale=ESCALE, bias=bias3[:])
                    else:
                        nc.scalar.activation(pt3[:, :, r:QB], sp3[:, :, r:QB],
                                             EXP, scale=ESCALE, bias=bias3[:])
                    if DBG and b == 0 and i == 0 and j == 0:
                        nc.scalar.dma_start(out=dbg_p[:], in_=pt[:])
                    pend.append((pt3, r, j))
                    if len(pend) > 2:
                        do_pv(False)
                    feed(1)
                while len(pend) > 1:
                    do_pv(False)
                do_pv(True)
                # softmax normalize + fp8 hi/lo split + A2A send staging
                for h in range(HPC):
                    rc = scp.tile([1, QB], F32, tag="recip")
                    nc.vector.reciprocal(rc[:], ya[h][64:65, :])
                    bc = scp.tile([64, QB], F32, tag="bcast")
                    nc.gpsimd.partition_broadcast(bc[:], rc[:])
                    yt = ysbp.tile([64, QB], F32, tag="yf32")
                    nc.vector.tensor_mul(yt[:], ya[h][0:64, :], bc[:])
                    yh8 = ysbp.tile([64, QB], F8, tag="yhi")
                    nc.vector.tensor_copy(yh8[:], yt[:])
                    yl8 = ysbp.tile([64, QB], F8, tag="ylo")
                    nc.gpsimd.tensor_sub(yl8[:], yt[:], yh8[:])
                    if DBG and b == 0 and i == 0 and h == 0:
                        nc.scalar.dma_start(out=dbg_yt[:], in_=yt[:])
                        nc.scalar.dma_start(out=dbg_yh[:], in_=yh8[:])
                        nc.scalar.dma_start(out=dbg_yl[:], in_=yl8[:])
                    # A2A splits sends into 8 contiguous 256-row chunks (one
                    # per dest core): rows = (m 8, l 2, p 128)
                    for li, src, qeng in ((0, yh8, nc.sync), (1, yl8, nc.scalar)):
                        for q in range(2):
                            base = (2 * i + q) * 256 + li * 128 + h * 64
                            qeng.dma_start(
                                out=sends[b][base : base + 64, :],
                                in_=src[:, q * 256 : (q + 1) * 256],
                            )

            def dump_sends(b):
                sdt = rvp.tile([128, 4096], F8, tag="sdbg")
                nc.sync.dma_start(
                    out=sdt[:].rearrange("p (c l t) -> p c l t", c=8, l=2),
                    in_=sends[b][:].rearrange("(c l p) t -> p c l t", c=8, l=2),
                )
                nc.sync.dma_start(out=dbg_sd[:], in_=sdt[:])

            def a2a(b):
                nc.gpsimd.collective_compute(
                    "AllToAll",
                    mybir.AluOpType.bypass,
                    replica_groups=[list(range(N_CORES))],
                    ins=[sends[b][:]],
                    outs=[recvs[b][:]],
                )

            rv_tiles = {}

            def load_rv(b):
                rv = rvp.tile([128, 2 * 8 * TPB], F8, tag="rv", name=f"rv{b}")
                rv_tiles[b] = rv
                nc.sync.dma_start(
                    out=rv[:].rearrange("p (c l t) -> p c l t", c=8, l=2),
                    in_=recvs[b][:].rearrange("(c l p) t -> p c l t", c=8, l=2),
                )

            def proj_gen(b):
                """Projection for this core's 256 tokens of batch b.
                Per c-tile: [yhi,ylo] x [wp_hi dup]; per c-pair: [yhi,yhi] x [wp_lo]."""
                rv = rv_tiles[b]
                if DBG and b == 0:
                    nc.scalar.dma_start(out=dbg_rv[:], in_=rv[:])
                rv4 = rv[:].rearrange("p (c l t) -> p c l t", c=8, l=2)
                wph4 = wp_hid_s[:].rearrange("k (c two f) -> k c two f", c=8, two=2)
                wpl3 = wp_lo_s[:].rearrange("k (c f) -> k c f", c=8)
                for tt in range(2):
                    ts = tt * 128
                    for fo in range(2):
                        pj = psp.tile([128, 512], F32, tag="ps", name="pj")
                        for fq in range(2):
                            fc = fo * 512 + fq * 256
                            freg = pj[:, fq * 256 : (fq + 1) * 256]
                            for c in range(8):
                                lhs = rv4[:, c, :, ts : ts + 128]  # [128, 2(l), 128]
                                rhs = wph4[:, c, :, fc : fc + 256]
                                nc.tensor.matmul(freg, lhs, rhs,
                                                 start=(c == 0 and fq == 0),
                                                 stop=False, perf_mode=DR)
                            for p in range(4):
                                lhs = rv4[:, 2 * p : 2 * p + 2, 0, ts : ts + 128]
                                rhs = wpl3[:, 2 * p : 2 * p + 2, fc : fc + 256]
                                nc.tensor.matmul(freg, lhs, rhs,
                                                 start=False, stop=(p == 3),
                                                 perf_mode=DR)
                            yield
                        ob = ysbp.tile([128, 512], F32, tag="ob")
                        nc.vector.tensor_scalar_mul(ob[:], pj[:], 1.0 / SC)
                        row = b * 256 + ts
                        nc.sync.dma_start(
                            out=out_ext[row : row + 128, fo * 512 : (fo + 1) * 512],
                            in_=ob[:],
                        )
                yield

            # ---------------- emission schedule ----------------
            from collections import deque

            class Feeder:
                def __init__(self):
                    self.q = deque()
                    self.added = 0
                    self.finished = 0

                def add(self, *gens):
                    self.q.extend(gens)
                    self.added += len(gens)
                    return self.added

                def feed(self, n=1):
                    done = 0
                    while self.q and done < n:
                        try:
                            next(self.q[0])
                            done += 1
                        except StopIteration:
                            self.q.popleft()
                            self.finished += 1
                    return done

                def drain_to(self, mark):
                    while self.finished < mark and self.q:
                        self.feed(64)

                def drain(self):
                    while self.feed(64):
                        pass

            feeder = Feeder()

            def phase1_gens(b):
                return [qkv_gen(b, tb) for tb in range(4)]

            def phase2(b):
                for i in range(NQB):
                    attn_unit(b, i, feeder.feed)
                if DBG and b == 0:
                    dump_sends(b)
                a2a(b)
                load_rv(b)

            m0 = feeder.add(*phase1_gens(0))
            feeder.drain_to(m0)
            load_wp()
            if DBG:
                nc.scalar.dma_start(out=dbg_q[:], in_=qT[:, 0:1024])
                nc.scalar.dma_start(out=dbg_k[:], in_=kT[:, 0:1024])
                nc.scalar.dma_start(out=dbg_va[:], in_=vaug[(0, 0)][:])
            feeder.add(*phase1_gens(1))
            m1 = feeder.added
            phase2(0)
            feeder.drain_to(m1)
            feeder.add(*phase1_gens(2))
            m2 = feeder.added
            phase2(1)
            feeder.drain_to(m2)
            feeder.add(*phase1_gens(3))
            m3 = feeder.added
            phase2(2)
            feeder.drain_to(m3)
            phase2(3)
            feeder.drain()
            for b in range(B):
                feeder.add(proj_gen(b))
            feeder.drain()

    nc.finalize()
    return nc


def _prep_inputs(x, W_attn, b_attn, W_proj, b_proj):
    x = np.asarray(x, dtype=np.float32)
    W_attn = np.asarray(W_attn, dtype=np.float32)
    W_proj = np.asarray(W_proj, dtype=np.float32)

    xT = np.ascontiguousarray(x.reshape(BT, C).T)          # [C, BT]
    xhi = xT.astype(E4)
    xlo = (xT - xhi.astype(np.float32)).astype(E4)

    def wsplit(Wt):                                        # Wt: [C, F] f32
        A = np.ascontiguousarray(SC * Wt, dtype=np.float32)
        hi = A.astype(E4)
        lo = (A - hi.astype(np.float32)).astype(E4)
        F = Wt.shape[1]
        # [C, F] -> [128, (c 8, F)]
        hi_l = hi.reshape(8, 128, F).transpose(1, 0, 2).reshape(128, 8 * F)
        lo_l = lo.reshape(8, 128, F).transpose(1, 0, 2).reshape(128, 8 * F)
        return hi_l, lo_l

    def dup(w, F):                                         # [128, 8*F] -> [128, 8*2*F]
        return np.ascontiguousarray(
            np.repeat(w.reshape(128, 8, 1, F), 2, axis=2).reshape(128, 16 * F))

    s = np.arange(128)[:, None]
    t = np.arange(128)[None, :]
    mska_np = (-1e9 * np.eye(128, dtype=np.float32)).astype(BF)
    mskb_np = (t < s).astype(np.float32).astype(BF)        # [p,t] = 1 if t < p

    wpT = W_proj.T                                         # [C, C]
    wp_hi_l, wp_lo_l = wsplit(wpT)

    in_maps = []
    for k in range(N_CORES):
        r0 = k * HPC * D                                   # 128*k
        wqh, wql = wsplit(W_attn[r0 : r0 + 128, :].T)
        wkh, wkl = wsplit(W_attn[C + r0 : C + r0 + 128, :].T)
        wvh, wvl = wsplit(W_attn[2 * C + r0 : 2 * C + r0 + 128, :].T)
        in_maps.append({
            "xhi": xhi, "xlo": xlo,
            "wq_hi": wqh, "wq_lo": wql,
            "wk_hi": wkh, "wk_lo": wkl,
            "wv_hid": dup(wvh, 128), "wv_lo": wvl,
            "wp_hid": dup(wp_hi_l, 1024), "wp_lo": wp_lo_l,
            "mska": mska_np, "mskb": mskb_np,
        })
    return in_maps


def kernel(x, W_attn, b_attn, W_proj, b_proj):
    global _PROGRAM, LAST_RESULTS
    if _PROGRAM is None:
        _PROGRAM = _build_program()
    nc = _PROGRAM

    in_maps = _prep_inputs(x, W_attn, b_attn, W_proj, b_proj)
    res = run_bass_kernel_spmd(nc, in_maps, list(range(N_CORES)), **RUN_KWARGS)
    LAST_RESULTS = res

    out = np.empty((B, T, C), dtype=np.float32)
    for k in range(N_CORES):
        ok = res.results[k]["out"]                         # [TOKS, C]
        for b in range(B):
            out[b, k * TPB : (k + 1) * TPB, :] = ok[b * TPB : (b + 1) * TPB, :]
    return out


# revision 21
# speedup vs baseline: 1.4089x; 1.1171x over previous
"""Causal self-attention (B=4, T=2048, C=1024, H=16) on 8 TRN2 NeuronCores.

Sharding: tensor-parallel over heads (2 heads/core) for QKV+attention;
data-parallel over tokens for the output projection, connected by one
AllToAll per batch (fp8 hi/lo payload).

fp8 DoubleRow strategy (cost model: DR = 0.5 cycles/moving-row, two
contraction tiles per instruction):
  - QKV q/k: 3-term DR over c-tile pairs:
      [Whi(c0),Whi(c1)]x[xhi(c0),xhi(c1)] + [Whi]x[xlo pair] + [Wlo]x[xhi pair]
    x is split hi/lo fp8 on the host; W pre-scaled by 32 (fp8 subnormal
    avoidance) and split hi/lo.  Result PSUM holds 32*q exactly-ish.
  - v: token-major (stationary x pair, moving Wv) -> vaug needs NO PE
    transposes; per c-tile: [xhi,xlo]x[Wvhi dup] + per c-pair [xhi,xhi]x[Wvlo].
  - S: bf16 (qT/kT evicted as 32q/32k bf16), full-rate 1 cycle/row.
  - causal mask: one PE matmul per diag block (-1e9*I stationary,
    strict-lower ones moving, broadcast over both heads).
  - exp: ACT, scale 2^-13 (=0.125/32^2), bias -3 (global softmax shift,
    exact), fp8 P out; queries 0:128 of the first diag tile use bias 0.
  - PV: per key-tile single DR: [vhi_j, vlo_j] x [P_j broadcast-dup]
    -> V exact, P fp8.  ya accumulates [65, 512] (65th row = 32-scaled
    ones column -> softmax sums).
  - proj: per c-tile [yhi,ylo]x[Wp_hi dup] + per c-pair [yhi,yhi]x[Wp_lo],
    y split hi/lo at eviction, A2A carries both fp8 halves.
Schedule: all projection work deferred to the end so proj(0..2) hides
A2A(3); output DMA'd straight from PSUM.
"""

import sys

sys.path.insert(0, "/opt/trn_rl_repo")

import numpy as np
import ml_dtypes

import concourse.bass as bass
import concourse.bacc as bacc
import concourse.mybir as mybir
import concourse.tile as tile
from concourse.bass_utils import run_bass_kernel_spmd

N_CORES = 8
B, T, C = 4, 2048, 1024
H, D = 16, 64
HPC = H // N_CORES          # 2 heads per core
BT = B * T                  # 8192 tokens
QB = 512                    # query block
SB = 128                    # key tile
NQB = T // QB               # 4 query blocks per batch
NSB = T // SB               # 16 key tiles per batch
TOKS = BT // N_CORES        # 1024 output tokens per core
TPB = 256                   # tokens per (core, batch)
SC = 32.0                   # weight/value scale (power of 2)
ESCALE = 0.125 / (SC * SC)  # exp scale on 32q.32k logits = 2^-13
SHIFT = 3.0                 # global softmax shift (exact for softmax)

F32 = mybir.dt.float32
BF16 = mybir.dt.bfloat16
F8 = mybir.dt.float8e4
EXP = mybir.ActivationFunctionType.Exp
DR = mybir.MatmulPerfMode.DoubleRow
E4 = ml_dtypes.float8_e4m3
BF = ml_dtypes.bfloat16

RUN_KWARGS: dict = {}
LAST_RESULTS = None
_PROGRAM = None


def _build_program():
    nc = bacc.Bacc(num_devices=N_CORES)

    xall = nc.declare_dram_parameter("xall", [2 * C, BT], F8, isOutput=False)
    wq_hi = nc.declare_dram_parameter("wq_hi", [128, 8 * 128], F8, isOutput=False)
    wq_lo = nc.declare_dram_parameter("wq_lo", [128, 8 * 128], F8, isOutput=False)
    wk_hi = nc.declare_dram_parameter("wk_hi", [128, 8 * 128], F8, isOutput=False)
    wk_lo = nc.declare_dram_parameter("wk_lo", [128, 8 * 128], F8, isOutput=False)
    wv_hid = nc.declare_dram_parameter("wv_hid", [128, 8 * 2 * 128], F8, isOutput=False)
    wv_lo = nc.declare_dram_parameter("wv_lo", [128, 8 * 128], F8, isOutput=False)
    wp_hid = nc.declare_dram_parameter("wp_hid", [128, 8 * 2 * 1024], F8, isOutput=False)
    wp_lo = nc.declare_dram_parameter("wp_lo", [128, 8 * 1024], F8, isOutput=False)
    mska = nc.declare_dram_parameter("mska", [128, 128], BF16, isOutput=False)  # -1e9*I
    mskb = nc.declare_dram_parameter("mskb", [128, 128], BF16, isOutput=False)  # [p,t]=1 if t<p
    out_ext = nc.declare_dram_parameter("out", [TOKS, C], F32, isOutput=True)
    import os
    DBG = bool(os.environ.get("BASS_DEBUG_DUMP"))
    if DBG:
        dbg_q = nc.declare_dram_parameter("dbg_q", [128, 1024], BF16, isOutput=True)
        dbg_k = nc.declare_dram_parameter("dbg_k", [128, 1024], BF16, isOutput=True)
        dbg_va = nc.declare_dram_parameter("dbg_va", [128, NSB * 256], F8, isOutput=True)
        dbg_p = nc.declare_dram_parameter("dbg_p", [128, 1024], F8, isOutput=True)
        dbg_yt = nc.declare_dram_parameter("dbg_yt", [64, 512], F32, isOutput=True)
        dbg_rv = nc.declare_dram_parameter("dbg_rv", [128, 4096], F8, isOutput=True)
        dbg_yh = nc.declare_dram_parameter("dbg_yh", [64, 512], F8, isOutput=True)
        dbg_yl = nc.declare_dram_parameter("dbg_yl", [64, 512], F8, isOutput=True)
        dbg_sd = nc.declare_dram_parameter("dbg_sd", [128, 4096], F8, isOutput=True)

    # A2A bounce buffers: rows 0:1024 = yhi (8 m-chunks x 128), 1024:2048 = ylo
    sends = [nc.dram_tensor(f"send{b}", [2 * N_CORES * 128, TPB], F8) for b in range(B)]
    recvs = [nc.dram_tensor(f"recv{b}", [2 * N_CORES * 128, TPB], F8) for b in range(B)]

    with tile.TileContext(nc) as tc:
        with (
            tc.tile_pool(name="const", bufs=1) as constp,
            tc.tile_pool(name="wgt", bufs=1) as wgtp,
            tc.tile_pool(name="qk", bufs=1) as qkp,
            tc.tile_pool(name="xt", bufs=3) as xtp,
            tc.tile_pool(name="vaug", bufs=4) as vaugp,
            tc.tile_pool(name="pp", bufs=7) as ppool,
            tc.tile_pool(name="sc", bufs=4) as scp,
            tc.tile_pool(name="ysb", bufs=4) as ysbp,
            tc.tile_pool(name="rv", bufs=4) as rvp,
            tc.tile_pool(name="ps", bufs=2, space="PSUM") as psp,     # qkv/v/proj (1 bank tiles)
            tc.tile_pool(name="sps", bufs=2, space="PSUM") as sps,    # S^T [128, 2*512] f32
            tc.tile_pool(name="yaps", bufs=2, space="PSUM") as yaps,  # ya [65, 512] f32
        ):
            # ---------------- constants + weights ----------------
            mska_s = constp.tile([128, 128], BF16, tag="mska")
            nc.scalar.dma_start(out=mska_s[:], in_=mska[:])
            mskb_s = constp.tile([128, 128], BF16, tag="mskb")
            nc.scalar.dma_start(out=mskb_s[:], in_=mskb[:])
            bias3 = constp.tile([128, 1], F32, tag="bias3")
            nc.gpsimd.memset(bias3[:], -SHIFT)

            wq_hi_s = wgtp.tile([128, 1024], F8, tag="wqh")
            wq_lo_s = wgtp.tile([128, 1024], F8, tag="wql")
            wk_hi_s = wgtp.tile([128, 1024], F8, tag="wkh")
            wk_lo_s = wgtp.tile([128, 1024], F8, tag="wkl")
            wv_hid_s = wgtp.tile([128, 2048], F8, tag="wvh")
            wv_lo_s = wgtp.tile([128, 1024], F8, tag="wvl")
            for dst, src in (
                (wq_hi_s, wq_hi), (wq_lo_s, wq_lo),
                (wk_hi_s, wk_hi), (wk_lo_s, wk_lo),
                (wv_hid_s, wv_hid), (wv_lo_s, wv_lo),
            ):
                nc.scalar.dma_start(out=dst[:], in_=src[:])

            # wp loaded lazily after phase1(0) is emitted
            wp_hid_s = wgtp.tile([128, 16384], F8, tag="wph")
            wp_lo_s = wgtp.tile([128, 8192], F8, tag="wpl")

            def load_wp():
                nc.scalar.dma_start(out=wp_hid_s[:], in_=wp_hid[:])
                nc.scalar.dma_start(out=wp_lo_s[:], in_=wp_lo[:])

            qT = qkp.tile([128, BT], BF16, tag="qT")   # 32*q, bf16
            kT = qkp.tile([128, BT], BF16, tag="kT")

            vaug = {}  # (b, h) -> [128, NSB*130] F8 (per key tile: hi 65 | lo 65)

            def qkv_gen(b, tb):
                """QKV for one 512-token block: q,k 3-term DR feature-major;
                v token-major (no transposes). Yields between PE chunks."""
                if tb == 0:
                    for h in range(HPC):
                        va = vaugp.tile([128, NSB * 256], F8, tag="vaug",
                                        name=f"va{b}_{h}")
                        vaug[(b, h)] = va
                        # per-j slot layout (256 wide): hi slot [v 0:64 | ones@64 |
                        # zero pad 65:128], lo slot [v 128:192 | zero pad 192:256]
                        # (zero pads keep the unused ya rows 65:127 finite)
                        va3 = va[:].rearrange("p (j c) -> p j c", j=NSB)
                        nc.gpsimd.memset(va3[:, :, 64:128], 0.0)
                        nc.gpsimd.memset(va3[:, :, 192:256], 0.0)
                        nc.gpsimd.memset(va3[:, :, 64:65], SC)
                base = b * T + tb * QB
                xt = xtp.tile([128, 2 * 8 * QB], F8, tag="xt")  # hi: c*512, lo: 4096+c*512
                nc.sync.dma_start(
                    out=xt[:].rearrange("p (l c t) -> p l c t", l=2, c=8),
                    in_=xall[:, base : base + QB].rearrange(
                        "(l c p) t -> p l c t", l=2, c=8),
                )
                yield
                # ---- q, k: 3-term DR over c-pairs, halves of 256 tokens ----
                # xt4[p, l, c, t]: l = hi/lo, c = 8 chunks, t = 512 tokens
                xt4 = xt[:].rearrange("p (l c t) -> p l c t", l=2, c=8)
                pq = psp.tile([128, QB], F32, tag="ps", name="pq")
                pk = psp.tile([128, QB], F32, tag="ps", name="pk")
                for out_ps, whi, wlo in ((pq, wq_hi_s, wq_lo_s), (pk, wk_hi_s, wk_lo_s)):
                    for hf in range(2):
                        ts = hf * 256
                        for p in range(4):
                            lw_hi = whi[:, p * 256 : (p + 1) * 256].rearrange(
                                "k (two m) -> k two m", two=2)
                            lw_lo = wlo[:, p * 256 : (p + 1) * 256].rearrange(
                                "k (two m) -> k two m", two=2)
                            # x pair APs: [128, (c pair), 256]
                            xh = xt4[:, 0, 2 * p : 2 * p + 2, ts : ts + 256]
                            xl = xt4[:, 1, 2 * p : 2 * p + 2, ts : ts + 256]
                            first = p == 0 and hf == 0
                            last = p == 3
                            nc.tensor.matmul(out_ps[:, ts : ts + 256], lw_hi, xh,
                                             start=first, stop=False, perf_mode=DR)
                            nc.tensor.matmul(out_ps[:, ts : ts + 256], lw_hi, xl,
                                             start=False, stop=False, perf_mode=DR)
                            nc.tensor.matmul(out_ps[:, ts : ts + 256], lw_lo, xh,
                                             start=False, stop=last, perf_mode=DR)
                        yield
                nc.vector.tensor_copy(qT[:, base : base + QB], pq[:])
                nc.vector.tensor_copy(kT[:, base : base + QB], pk[:])
                yield
                # ---- v: token-major, 4 key tiles of 128 tokens ----
                # per c-tile: lhsT = (xhi(c), xlo(c)) pair -> exact x; rhs = Wv_hi dup
                # per c-pair: lhsT = (xhi(c0), xhi(c1)); rhs = (Wv_lo(c0), Wv_lo(c1))
                wv8 = wv_lo_s[:].rearrange("k (c m) -> k c m", c=8)
                pv = psp.tile([128, 4 * 128], F32, tag="ps", name="pv")  # (tok-tile, feat)
                for tt in range(2):  # two double-tile chunks to bound inst burst
                    for ttt in range(2):
                        t4 = tt * 2 + ttt
                        ts = t4 * 128
                        reg = pv[:, t4 * 128 : (t4 + 1) * 128]
                        for c in range(8):
                            xpair = xt4[:, :, c, ts : ts + 128]  # [128, 2(l), 128]
                            wvd = wv_hid_s[:, c * 256 : (c + 1) * 256].rearrange(
                                "k (two m) -> k two m", two=2)
                            nc.tensor.matmul(reg, xpair, wvd,
                                             start=(c == 0 and t4 == 0),
                                             stop=False, perf_mode=DR)
                        for p in range(4):
                            xhp = xt4[:, 0, 2 * p : 2 * p + 2, ts : ts + 128]
                            wvl = wv8[:, 2 * p : 2 * p + 2, :]
                            nc.tensor.matmul(reg, xhp, wvl,
                                             start=False, stop=(p == 3), perf_mode=DR)
                    yield
                # evictions: vhi (DVE copy), vlo (DVE mixed sub)
                for t4 in range(4):
                    j = tb * 4 + t4
                    reg = pv[:, t4 * 128 : (t4 + 1) * 128]
                    for h in range(HPC):
                        va = vaug[(b, h)]
                        hslice = reg[:, h * 64 : (h + 1) * 64]
                        nc.vector.tensor_copy(va[:, j * 256 : j * 256 + 64], hslice)
                        nc.vector.tensor_sub(va[:, j * 256 + 128 : j * 256 + 192],
                                             hslice, va[:, j * 256 : j * 256 + 64])
                yield

            def attn_unit(b, i, feed):
                """One query block: S (bf16) + PE mask + exp (fp8, shift) +
                PV single-tile DR + softmax normalize + A2A send staging."""
                ya = [yaps.tile([128, QB], F32, tag="yaug", name=f"ya{h}")
                      for h in range(HPC)]
                jmax = 4 * (i + 1)
                pend = []
                emitted = [[] for _ in range(HPC)]  # (j, r) per head, for flags

                def do_pv(last):
                    # PSUM start=True poisons the whole 2KB bank (pending-zero),
                    # so ONLY the very first inst per ya bank may set it; later
                    # insts overwrite-on-first-touch of still-pending bytes.
                    pt3, r, j = pend.pop(0)
                    for h in range(HPC):
                        va = vaug[(b, h)]
                        lhs = va[:, j * 256 : j * 256 + 256].rearrange(
                            "p (two m) -> p two m", two=2)
                        a = r
                        while a < QB:
                            bnd = min(a + 256, QB)
                            rhs = pt3[:, h, a:bnd].unsqueeze(1).broadcast_to(
                                [128, 2, bnd - a])
                            is_last = last and bnd == QB
                            nc.tensor.matmul(ya[h][:, a:bnd], lhs, rhs,
                                             start=(j == 0 and a == 0),
                                             stop=is_last, perf_mode=DR)
                            a = bnd

                for j in range(jmax):
                    diag = j >= 4 * i
                    r = SB * j - QB * i if diag else 0
                    sp = sps.tile([128, 2 * QB], F32, tag="sp")
                    sp3 = sp[:].rearrange("p (h t) -> p h t", h=2)
                    for h in range(HPC):
                        nc.tensor.matmul(
                            sp3[:, h, r:QB],
                            kT[h * 64 : (h + 1) * 64, b * T + j * SB : b * T + (j + 1) * SB],
                            qT[h * 64 : (h + 1) * 64, b * T + i * QB + r : b * T + (i + 1) * QB],
                            start=True, stop=True,
                        )
                    if diag:
                        rhsm = mskb_s[:].unsqueeze(1).broadcast_to([128, 2, 128])
                        outm = sp3[:, :, r : r + 128]
                        nc.tensor.matmul(outm, mska_s[:], rhsm, start=False, stop=True,
                                         skip_group_check=True)
                    pt = ppool.tile([128, 2 * QB], F8, tag="P")
                    pt3 = pt[:].rearrange("p (h t) -> p h t", h=2)
                    if i == 0 and j == 0:
                        nc.scalar.activation(pt3[:, :, 0:128], sp3[:, :, 0:128],
                                             EXP, scale=ESCALE, bias=0.0)
                        nc.scalar.activation(pt3[:, :, 128:QB], sp3[:, :, 128:QB],
                                             EXP, scale=ESCALE, bias=bias3[:])
                    else:
                        nc.scalar.activation(pt3[:, :, r:QB], sp3[:, :, r:QB],
                                             EXP, scale=ESCALE, bias=bias3[:])
                    if DBG and b == 0 and i == 0 and j == 0:
                        nc.scalar.dma_start(out=dbg_p[:], in_=pt[:])
                    pend.append((pt3, r, j))
                    if len(pend) > 4:
                        do_pv(False)
                    feed(1)
                while len(pend) > 1:
                    do_pv(False)
                do_pv(True)
                # softmax normalize + fp8 hi/lo split + A2A send staging
                for h in range(HPC):
                    rc = scp.tile([1, QB], F32, tag="recip")
                    nc.vector.reciprocal(rc[:], ya[h][64:65, :])
                    bc = scp.tile([64, QB], F32, tag="bcast")
                    nc.gpsimd.partition_broadcast(bc[:], rc[:])
                    yt = ysbp.tile([64, QB], F32, tag="yf32")
                    nc.vector.tensor_mul(yt[:], ya[h][0:64, :], bc[:])
                    yh8 = ysbp.tile([64, QB], F8, tag="yhi")
                    nc.vector.tensor_copy(yh8[:], yt[:])
                    yl8 = ysbp.tile([64, QB], F8, tag="ylo")
                    nc.gpsimd.tensor_sub(yl8[:], yt[:], yh8[:])
                    if DBG and b == 0 and i == 0 and h == 0:
                        nc.scalar.dma_start(out=dbg_yt[:], in_=yt[:])
                        nc.scalar.dma_start(out=dbg_yh[:], in_=yh8[:])
                        nc.scalar.dma_start(out=dbg_yl[:], in_=yl8[:])
                    # A2A splits sends into 8 contiguous 256-row chunks (one
                    # per dest core): rows = (m 8, l 2, p 128)
                    dst4 = sends[b][:].rearrange("(m l p) t -> m l p t", l=2, p=128)
                    for li, src in ((0, yh8), (1, yl8)):
                        sl = dst4[2 * i : 2 * i + 2, li, h * 64 : (h + 1) * 64, :]
                        nc.sync.dma_start(
                            out=sl.transpose([1, 0, 2]),
                            in_=src[:].rearrange("p (m t) -> p m t", m=2),
                        )

            def dump_sends(b):
                sdt = rvp.tile([128, 4096], F8, tag="sdbg")
                nc.sync.dma_start(
                    out=sdt[:].rearrange("p (c l t) -> p c l t", c=8, l=2),
                    in_=sends[b][:].rearrange("(c l p) t -> p c l t", c=8, l=2),
                )
                nc.sync.dma_start(out=dbg_sd[:], in_=sdt[:])

            def a2a(b):
                nc.gpsimd.collective_compute(
                    "AllToAll",
                    mybir.AluOpType.bypass,
                    replica_groups=[list(range(N_CORES))],
                    ins=[sends[b][:]],
                    outs=[recvs[b][:]],
                )

            rv_tiles = {}

            def load_rv(b):
                rv = rvp.tile([128, 2 * 8 * TPB], F8, tag="rv", name=f"rv{b}")
                rv_tiles[b] = rv
                nc.sync.dma_start(
                    out=rv[:].rearrange("p (c l t) -> p c l t", c=8, l=2),
                    in_=recvs[b][:].rearrange("(c l p) t -> p c l t", c=8, l=2),
                )

            def proj_gen(b):
                """Projection for this core's 256 tokens of batch b.
                Per c-tile: [yhi,ylo] x [wp_hi dup]; per c-pair: [yhi,yhi] x [wp_lo]."""
                rv = rv_tiles[b]
                if DBG and b == 0:
                    nc.scalar.dma_start(out=dbg_rv[:], in_=rv[:])
                rv4 = rv[:].rearrange("p (c l t) -> p c l t", c=8, l=2)
                wph4 = wp_hid_s[:].rearrange("k (c two f) -> k c two f", c=8, two=2)
                wpl3 = wp_lo_s[:].rearrange("k (c f) -> k c f", c=8)
                for tt in range(2):
                    ts = tt * 128
                    for fo in range(2):
                        pj = psp.tile([128, 512], F32, tag="ps", name="pj")
                        for fq in range(2):
                            fc = fo * 512 + fq * 256
                            freg = pj[:, fq * 256 : (fq + 1) * 256]
                            for c in range(8):
                                lhs = rv4[:, c, :, ts : ts + 128]  # [128, 2(l), 128]
                                rhs = wph4[:, c, :, fc : fc + 256]
                                nc.tensor.matmul(freg, lhs, rhs,
                                                 start=(c == 0 and fq == 0),
                                                 stop=False, perf_mode=DR)
                            for p in range(4):
                                lhs = rv4[:, 2 * p : 2 * p + 2, 0, ts : ts + 128]
                                rhs = wpl3[:, 2 * p : 2 * p + 2, fc : fc + 256]
                                nc.tensor.matmul(freg, lhs, rhs,
                                                 start=False, stop=(p == 3),
                                                 perf_mode=DR)
                            yield
                        ob = ysbp.tile([128, 512], F32, tag="ob")
                        nc.vector.tensor_scalar_mul(ob[:], pj[:], 1.0 / SC)
                        row = b * 256 + ts
                        nc.sync.dma_start(
                            out=out_ext[row : row + 128, fo * 512 : (fo + 1) * 512],
                            in_=ob[:],
                        )
                yield

            # ---------------- emission schedule ----------------
            from collections import deque

            class Feeder:
                def __init__(self):
                    self.q = deque()
                    self.added = 0
                    self.finished = 0

                def add(self, *gens):
                    self.q.extend(gens)
                    self.added += len(gens)
                    return self.added

                def feed(self, n=1):
                    done = 0
                    while self.q and done < n:
                        try:
                            next(self.q[0])
                            done += 1
                        except StopIteration:
                            self.q.popleft()
                            self.finished += 1
                    return done

                def drain_to(self, mark):
                    while self.finished < mark and self.q:
                        self.feed(64)

                def drain(self):
                    while self.feed(64):
                        pass

            feeder = Feeder()

            def phase1_gens(b):
                return [qkv_gen(b, tb) for tb in range(4)]

            def phase2(b, rv=True):
                for i in range(NQB):
                    attn_unit(b, i, feeder.feed)
                if DBG and b == 0:
                    dump_sends(b)
                a2a(b)
                if rv:
                    load_rv(b)

            m0 = feeder.add(*phase1_gens(0))
            feeder.drain_to(m0)
            load_wp()
            if DBG:
                nc.scalar.dma_start(out=dbg_q[:], in_=qT[:, 0:1024])
                nc.scalar.dma_start(out=dbg_k[:], in_=kT[:, 0:1024])
                nc.scalar.dma_start(out=dbg_va[:], in_=vaug[(0, 0)][:])
            feeder.add(*phase1_gens(1))
            m1 = feeder.added
            phase2(0)
            feeder.drain_to(m1)
            feeder.add(*phase1_gens(2))
            m2 = feeder.added
            phase2(1)
            feeder.drain_to(m2)
            feeder.add(*phase1_gens(3))
            m3 = feeder.added
            phase2(2)
            feeder.drain_to(m3)
            # proj(0..2) ride as PE fillers through batch-3 attention and
            # cover the A2A(3) window; load_rv(3) is emitted after them so
            # their DMA-queue waits don't conservatively include rv3.
            feeder.add(proj_gen(0), proj_gen(1), proj_gen(2))
            phase2(3, rv=False)
            feeder.drain()
            load_rv(3)
            feeder.add(proj_gen(3))
            feeder.drain()

    nc.finalize()
    return nc


def _prep_inputs(x, W_attn, b_attn, W_proj, b_proj):
    x = np.asarray(x, dtype=np.float32)
    W_attn = np.asarray(W_attn, dtype=np.float32)
    W_proj = np.asarray(W_proj, dtype=np.float32)

    xT = np.ascontiguousarray(x.reshape(BT, C).T)          # [C, BT]
    xhi = xT.astype(E4)
    xlo = (xT - xhi.astype(np.float32)).astype(E4)
    xall = np.ascontiguousarray(np.concatenate([xhi, xlo], axis=0))  # [2C, BT]

    def wsplit(Wt):                                        # Wt: [C, F] f32
        A = np.ascontiguousarray(SC * Wt, dtype=np.float32)
        hi = A.astype(E4)
        lo = (A - hi.astype(np.float32)).astype(E4)
        F = Wt.shape[1]
        # [C, F] -> [128, (c 8, F)]
        hi_l = hi.reshape(8, 128, F).transpose(1, 0, 2).reshape(128, 8 * F)
        lo_l = lo.reshape(8, 128, F).transpose(1, 0, 2).reshape(128, 8 * F)
        return hi_l, lo_l

    def dup(w, F):                                         # [128, 8*F] -> [128, 8*2*F]
        return np.ascontiguousarray(
            np.repeat(w.reshape(128, 8, 1, F), 2, axis=2).reshape(128, 16 * F))

    s = np.arange(128)[:, None]
    t = np.arange(128)[None, :]
    mska_np = (-1e9 * np.eye(128, dtype=np.float32)).astype(BF)
    mskb_np = (t < s).astype(np.float32).astype(BF)        # [p,t] = 1 if t < p

    wpT = W_proj.T                                         # [C, C]
    wp_hi_l, wp_lo_l = wsplit(wpT)

    in_maps = []
    for k in range(N_CORES):
        r0 = k * HPC * D                                   # 128*k
        wqh, wql = wsplit(W_attn[r0 : r0 + 128, :].T)
        wkh, wkl = wsplit(W_attn[C + r0 : C + r0 + 128, :].T)
        wvh, wvl = wsplit(W_attn[2 * C + r0 : 2 * C + r0 + 128, :].T)
        in_maps.append({
            "xall": xall,
            "wq_hi": wqh, "wq_lo": wql,
            "wk_hi": wkh, "wk_lo": wkl,
            "wv_hid": dup(wvh, 128), "wv_lo": wvl,
            "wp_hid": dup(wp_hi_l, 1024), "wp_lo": wp_lo_l,
            "mska": mska_np, "mskb": mskb_np,
        })
    return in_maps


def kernel(x, W_attn, b_attn, W_proj, b_proj):
    global _PROGRAM, LAST_RESULTS
    if _PROGRAM is None:
        _PROGRAM = _build_program()
    nc = _PROGRAM

    in_maps = _prep_inputs(x, W_attn, b_attn, W_proj, b_proj)
    res = run_bass_kernel_spmd(nc, in_maps, list(range(N_CORES)), **RUN_KWARGS)
    LAST_RESULTS = res

    out = np.empty((B, T, C), dtype=np.float32)
    for k in range(N_CORES):
        ok = res.results[k]["out"]                         # [TOKS, C]
        for b in range(B):
            out[b, k * TPB : (k + 1) * TPB, :] = ok[b * TPB : (b + 1) * TPB, :]
    return out
